# revision 43
# baseline (speedup 1.0000x reference)
"""Trainium2 Bass kernel for CrossNonLocalBlock (v4).

Shapes (hardcoded): B=8, Cs=Ct=256, Ci=128, H=W=64 (N=4096 spatial).
Sharding: data-parallel over batch (1 batch element per NeuronCore, 8 cores);
1x1-conv / BN params replicated; BN batch statistics all-reduced in-kernel.

Per-core dataflow, engine-balanced around the ACT-bound softmax exp
(4096x4096 logits -> 16.8M exp/core = ~110us of ACT at 1.2GHz):

  DMA (serial ~330GB/s fabric, strict order): one packed param tensor ->
    x[:, 0:512] -> l (full) -> x rest -> xB.  The loop is gated only on
    phi (l) + the first theta/g tiles (x quarter 0).
  head-1: theta chunk 0, gT tiles 0-3 (batched 4-per-PSUM-slot), phi.
    theta bias fused into copies; phi bias dropped (constant-in-m logit
    terms cancel in softmax).
  loop over 32 n-tiles; remaining theta chunks / gT tile groups trail
  inside iters 0..9 (copies on DVE/Pool, never ACT; PSUM borrowed from
  the S-staging pool):
    S_c = theta_nt^T phi_c          (PE->PSUM fp32r, 4x[128,1024])
    f_c = exp(S_c - SHIFT)          (ACT->SBUF bf16, no accum_out)
    Z   = rowsum(f) via DVE tensor_scalar+accum (4x perf mode)
    g'_nt = gT_nt / Z               (DVE, in-place bf16)
    fstore_nt = f[:,2048:] / Z      (Pool, normalized fp8e4)
    y0[:,0:2048] += g'_nt^T f       (PE, PSUM-resident, software-pipelined
                                     one iteration behind)
  phase2: y1 = sum_nt g8_nt^T fstore_nt (PE fp8 DoubleRow, 0.5cyc/row)
    interleaved with wy chunks: PE conv + ACT Identity(bias=w_b,
    accum_out->S1), S2 via DVE tensor_tensor_reduce chains;
    AllReduce([S1|S2]).
  tail: recompute wy, out = (wy-mean)*rstd*gamma+beta + l (l resident),
    pipelined ACT/DVE/Pool/DMA.
"""

import os
import sys

import numpy as np

if "/opt/trn_rl_repo" not in sys.path:
    sys.path.insert(0, "/opt/trn_rl_repo")

B, CS, CT, CI, N = 8, 256, 256, 128, 4096
NT = N // 128          # 32 n-tiles
M0 = 2048              # m-columns accumulated in PSUM during the n-loop
M1 = N - M0            # m-columns stored normalized fp8 for the 2nd pass
SHIFT = 50.0           # global logit shift fed to exp() as ACT bias
FP8G = 128.0           # fstore gain: keeps f*G/Z in fp8e4's normal range
BN_EPS = 1e-5
N_CORES = 8

# packed param layout (columns of the [128, PP] tensor)
_THW, _PHW, _GW, _WW = 0, 256, 512, 768
_THB, _WB, _GAM, _BET = 1024, 1025, 1027, 1029
PP = 1031

_CACHE = {}


def _build(n_cores: int, no_collective: bool = False):
    import concourse.bass as bass
    import concourse.mybir as mybir
    import concourse.tile as tile
    from concourse import bacc

    f32 = mybir.dt.float32
    f32r = mybir.dt.float32r
    bf16 = mybir.dt.bfloat16
    f8 = mybir.dt.float8e4
    f16 = mybir.dt.float16
    AF = mybir.ActivationFunctionType
    AX = mybir.AxisListType
    ALU = mybir.AluOpType
    DR = mybir.MatmulPerfMode.DoubleRow

    nc = bacc.Bacc("TRN2", target_bir_lowering=False, debug=False,
                   num_devices=n_cores)

    # ---- DRAM I/O (per-core) ----
    x = nc.dram_tensor("x", [CS, N], f32, kind="ExternalInput").ap()
    lres = nc.dram_tensor("lres", [CT, N], f32, kind="ExternalInput").ap()
    pk_d = nc.dram_tensor("pk", [128, PP], f32, kind="ExternalInput").ap()
    gb_d = nc.dram_tensor("g_b_row", [1, CI], f32, kind="ExternalInput").ap()
    out = nc.dram_tensor("out", [CT, N], f32, kind="ExternalOutput").ap()
    dbg = {}
    if os.environ.get("KDBG"):
        dbg["theta"] = nc.dram_tensor("d_theta", [CI, N], bf16,
                                      kind="ExternalOutput").ap()
        dbg["phi"] = nc.dram_tensor("d_phi", [CI, N], bf16,
                                    kind="ExternalOutput").ap()
        dbg["gts"] = nc.dram_tensor("d_gts", [128, NT * CI], bf16,
                                    kind="ExternalOutput").ap()
        dbg["g8"] = nc.dram_tensor("d_g8", [128, NT * CI], f8,
                                   kind="ExternalOutput").ap()
        dbg["fstore"] = nc.dram_tensor("d_fstore", [128, NT * M1], f8,
                                       kind="ExternalOutput").ap()
        dbg["ysb"] = nc.dram_tensor("d_ysb", [CI, N], bf16,
                                    kind="ExternalOutput").ap()
        dbg["stats"] = nc.dram_tensor("d_stats", [128, 4], f32,
                                      kind="ExternalOutput").ap()

    def r(ap):
        return ap.bitcast(f32r)

    with tile.TileContext(nc) as tc:
        with tc.tile_pool(name="persist", bufs=1) as pp:
            theta = pp.tile([CI, N], f32r)       # 16KB/part (rounded f32)
            phi = pp.tile([CI, N], f32r)         # 16KB/part
            gts = pp.tile([128, NT * CI], bf16)  # gT tiles (later scaled g') 8KB
            g8 = pp.tile([128, NT * CI], f8)     # unscaled gT in fp8       4KB
            lb0 = pp.tile([128, N], bf16)        # l residual, bf16         8KB
            lb1 = pp.tile([128, N], bf16)        # 8KB
            ysb = pp.tile([CI, N], bf16)         # attention out yT         8KB
            pk = pp.tile([128, PP], f32)         # packed params            4KB
            gwb = pp.tile([128, 2 * CI], f16)
            thwb = pp.tile([128, 2 * CI], f16)
            phwr = pp.tile([128, 2 * CI], f32r)
            wwb = pp.tile([CI, CT], bf16)
            gbr = pp.tile([1, CI], f32)
            gbrb = pp.tile([1, CI], f16)
            onesb = pp.tile([1, 128], f16)
            negshift = pp.tile([128, 1], f32)
            epsb = pp.tile([128, 1], f32)
            stats = pp.tile([128, 4], f32)       # [S1h0 S1h1 S2h0 S2h1]
            statsg = pp.tile([128, 4], f32)
            s1c = pp.tile([128, 8], f32)
            s2c = pp.tile([128, 8], f32)

            thw = pk[:, _THW:_THW + 256]
            phw = pk[:, _PHW:_PHW + 256]
            gw = pk[:, _GW:_GW + 256]
            ww = pk[:, _WW:_WW + 256]
            thb = pk[:, _THB:_THB + 1]
            wb = pk[:, _WB:_WB + 2]
            gam = pk[:, _GAM:_GAM + 2]
            bet = pk[:, _BET:_BET + 2]

            nc.sync.dma_start(pk[:, :], pk_d[:, :])
            nc.sync.dma_start(gbr[:, :], gb_d[:, :])
            nc.vector.memset(onesb[:, :], 1.0)
            nc.vector.memset(negshift[:, :], -SHIFT)
            nc.vector.memset(epsb[:, :], BN_EPS)
            # preload the exp ACT table while the head DMAs stream
            nc.scalar.activation(stats[:, 0:1], negshift[:, :], AF.Exp,
                                 scale=1.0)
            nc.vector.tensor_copy(gwb[:, :], gw)
            nc.vector.tensor_copy(thwb[:, :], thw)
            nc.vector.tensor_copy(phwr[:, :], phw)
            nc.vector.tensor_copy(gbrb[:, :], gbr[:, :])
            nc.vector.tensor_copy(wwb[:, :], ww)

            with tc.tile_pool(name="dram", bufs=1, space="DRAM") as dp:
                cc_in = dp.tile([128, 4], f32)
                cc_out = dp.tile([128, 4], f32,
                                 addr_space="Shared" if n_cores > 1 else "Local")

                # bulk loads: x quarter0 -> l -> x rest -> xB
                xB, xb16B, free_xB = [], [], []
                xA, xb16A, free_xA = [], [], []
                for k in range(2):
                    t, fr = tc.tile([128, M1], f32, name=f"xB{k}")
                    xB.append(t)
                    free_xB.append(fr)
                    t, fr = tc.tile([128, M1], f16, name=f"xb16B{k}")
                    xb16B.append(t)
                    free_xB.append(fr)
                for k in range(2):
                    t, fr = tc.tile([128, M0], f32, name=f"xA{k}")
                    xA.append(t)
                    free_xA.append(fr)
                    t, fr = tc.tile([128, M0], f16, name=f"xb16A{k}")
                    xb16A.append(t)
                    free_xA.append(fr)
                l0, free_l0 = tc.tile([128, N], f32r, name="l0")
                l1, free_l1 = tc.tile([128, N], f32r, name="l1")
                lt0, free_lt0 = tc.tile([128, N], f32, name="lt0")
                lt1, free_lt1 = tc.tile([128, N], f32, name="lt1")
                nc.sync.dma_start(lt0[:, :], lres[0:128, :])
                for k in range(2):
                    nc.sync.dma_start(xA[k][:, 0:512], x[k * 128:(k + 1) * 128, 0:512])
                nc.sync.dma_start(lt1[:, :], lres[128:256, :])
                for k in range(2):
                    nc.sync.dma_start(xA[k][:, 512:M0],
                                      x[k * 128:(k + 1) * 128, 512:M0])
                for k in range(2):
                    nc.sync.dma_start(xB[k][:, :], x[k * 128:(k + 1) * 128, M0:N])
                # round l to f32r (phi's matmul operand) in the DMA shadow;
                # l1 split ACT/DVE so phi-k1 starts right behind it
                for c in range(4):
                    sl = slice(c * 1024, (c + 1) * 1024)
                    nc.scalar.activation(l0[:, sl], lt0[:, sl], AF.Identity,
                                         scale=1.0)
                nc.vector.tensor_copy(lb0[:, :], lt0[:, :])
                for c in range(4):
                    sl = slice(c * 1024, (c + 1) * 1024)
                    if c % 2 == 0:
                        nc.scalar.activation(l1[:, sl], lt1[:, sl], AF.Identity,
                                             scale=1.0)
                    else:
                        nc.vector.tensor_copy(l1[:, sl], lt1[:, sl])

                def theta_chunk(ps, c, xk, base, on_act):
                    pt = ps.tile([128, 1024], f32, tag="s", name=f"thps{base + c}")
                    sl = slice(base + c * 512, base + (c + 1) * 512)
                    for k in range(2):
                        nc.tensor.matmul(pt[:, 0:512],
                                         thwb[:, k * CI:(k + 1) * CI],
                                         xk[k][:, c * 512:(c + 1) * 512],
                                         start=(k == 0), stop=(k == 1))
                    if on_act:
                        nc.scalar.activation(theta[:, sl], pt[:, 0:512],
                                             AF.Identity, bias=thb, scale=1.0)
                    else:
                        nc.vector.tensor_scalar(theta[:, sl], pt[:, 0:512],
                                                thb, None, op0=ALU.add)

                def g_group(ps, grp, xk, on_act):
                    # 4 gT tiles batched into one PSUM slot, two wide copies
                    pg = ps.tile([128, 1024], f32, tag="s", name=f"gps{grp}")
                    for j in range(4):
                        nt = grp * 4 + j
                        nsl = slice((nt * 128) % M0, (nt * 128) % M0 + 128)
                        osl = slice(j * 128, (j + 1) * 128)
                        nc.tensor.matmul(pg[:, osl], xk[0][:, nsl], gwb[:, 0:CI],
                                         start=True, stop=False)
                        nc.tensor.matmul(pg[:, osl], xk[1][:, nsl],
                                         gwb[:, CI:2 * CI], start=False, stop=False)
                        nc.tensor.matmul(pg[:, osl], onesb[:, :], gbrb[:, :],
                                         start=False, stop=True)
                    gsl = slice(grp * 512, (grp + 1) * 512)
                    if on_act:
                        nc.scalar.activation(gts[:, gsl], pg[:, 0:512],
                                             AF.Identity, scale=1.0)
                        nc.scalar.activation(g8[:, gsl], gts[:, gsl],
                                             AF.Identity, scale=1.0)
                    else:
                        nc.vector.tensor_copy(gts[:, gsl], pg[:, 0:512])

                # ---- head-1 (own PSUM pool, closed before the loop) ----
                with tc.tile_pool(name="ps0", bufs=4, space="PSUM") as ps0:
                    for k in range(2):
                        nc.vector.tensor_copy(xb16A[k][:, 0:512], xA[k][:, 0:512])
                    theta_chunk(ps0, 0, xb16A, 0, True)
                    g_group(ps0, 0, xb16A, True)
                    # phi: ALL k0 partials first (run in l1's DMA shadow,
                    # re-emitted once as a PE p-state bridge), then all k1.
                    # Copies split ACT/DVE to shorten the loop-gating drain.
                    pts = []
                    for c in range(4):
                        pts.append(ps0.tile([128, 1024], f32, tag="s",
                                            name=f"phps{c}"))
                    # p-state bridge: complete throwaway groups first
                    for rep in range(2):
                        for c in range(4):
                            if rep == 1 and c >= 2:
                                continue
                            for h in range(2):
                                sl = slice(c * 1024 + h * 512,
                                           c * 1024 + (h + 1) * 512)
                                nc.tensor.matmul(
                                    pts[c][:, h * 512:(h + 1) * 512],
                                    phwr[:, 0:CI], l0[:, sl],
                                    start=True, stop=True)
                    for c in range(4):
                        for h in range(2):
                            sl = slice(c * 1024 + h * 512,
                                       c * 1024 + (h + 1) * 512)
                            nc.tensor.matmul(
                                pts[c][:, h * 512:(h + 1) * 512],
                                phwr[:, 0:CI], l0[:, sl],
                                start=True, stop=False)
                    nc.vector.tensor_copy(lb1[:, :], lt1[:, :])
                    for c in range(4):
                        for h in range(2):
                            sl = slice(c * 1024 + h * 512, c * 1024 + (h + 1) * 512)
                            nc.tensor.matmul(
                                pts[c][:, h * 512:(h + 1) * 512],
                                phwr[:, CI:2 * CI], l1[:, sl],
                                start=False, stop=True)
                        psl = slice(c * 1024, (c + 1) * 1024)
                        if c % 2 == 0:
                            nc.scalar.activation(phi[:, psl], pts[c][:, :],
                                                 AF.Identity, scale=1.0)
                        else:
                            nc.vector.tensor_copy(phi[:, psl], pts[c][:, :])
                free_lt1()
                free_lt0()
                free_l1()
                free_l0()

                # ---- attention n-loop (fstore outlives the loop pools) ----
                with tc.tile_pool(name="fstore", bufs=1) as fsp:
                  fstore = fsp.tile([128, NT * M1], f8)   # 64KB/part
                  with tc.tile_pool(name="psS", bufs=2, space="PSUM") as psS, \
                       tc.tile_pool(name="psY0", bufs=1, space="PSUM") as psY0, \
                       tc.tile_pool(name="zp", bufs=2) as zp, \
                       tc.tile_pool(name="fwp", bufs=2) as fwp, \
                       tc.tile_pool(name="ftp", bufs=2) as ftp:
                    y0 = psY0.tile([CI, M0], f32)
                    prev = None  # (g_nt ap, fw tile) pending y0 matmuls

                    def pre_trailing(nt):
                        # pure-DVE feed copies (no PSUM involvement)
                        if nt in (0, 1, 2):   # xb16A rest: 2x[128,512] per iter
                            for k in range(2):
                                sl = slice(512 + nt * 512, 1024 + nt * 512)
                                nc.gpsimd.tensor_copy(xb16A[k][:, sl],
                                                      xA[k][:, sl])
                        if 3 <= nt <= 6:      # xb16B k0/k1 in [128,1024] pieces
                            k, q = (nt - 3) % 2, (nt - 3) // 2
                            sl = slice(q * 1024, (q + 1) * 1024)
                            nc.gpsimd.tensor_copy(xb16B[k][:, sl], xB[k][:, sl])

                    def trailing(nt):
                        # ONE PSUM-slot borrower per iteration, emitted at the
                        # iteration end so its slot-freeing copy has a whole
                        # iteration of slack before the slot is needed again.
                        # gT group j lands at iter 2j-1 (needed at iter 4j);
                        # theta chunk c of each half interleaves on even iters.
                        if nt % 2 == 1 and 1 <= nt <= 13:    # gT groups 1-7
                            grp = (nt + 1) // 2
                            g_group(psS, grp, xb16A if grp < 4 else xb16B, False)
                        elif nt in (2, 4, 6):                # theta-A chunks 1-3
                            theta_chunk(psS, nt // 2, xb16A, 0, False)
                        elif nt in (8, 10, 12, 14):          # theta-B chunks
                            theta_chunk(psS, (nt - 8) // 2, xb16B, M0, False)
                        if nt % 2 == 0 and 2 <= nt <= 14:
                            # fp8 copy of the group finished last iteration,
                            # before any of its tiles get scaled in-place
                            grp = nt // 2
                            gsl = slice(grp * 512, (grp + 1) * 512)
                            nc.vector.tensor_copy(g8[:, gsl], gts[:, gsl])

                    for nt in range(NT):
                        pre_trailing(nt)
                        th_nt = theta[:, nt * 128:(nt + 1) * 128]
                        fw = fwp.tile([128, M0], bf16, tag="fw", name=f"fw{nt}")
                        ft = ftp.tile([128, M1], bf16, tag="ft", name=f"ft{nt}")
                        for c in range(4):
                            sp = psS.tile([128, 1024], f32, tag="s")
                            for h in range(2):
                                sl = slice(c * 1024 + h * 512,
                                           c * 1024 + (h + 1) * 512)
                                nc.tensor.matmul(sp[:, h * 512:(h + 1) * 512],
                                                 th_nt, phi[:, sl],
                                                 start=True, stop=True)
                            if c < 2:
                                dst = fw[:, c * 1024:(c + 1) * 1024]
                            else:
                                dst = ft[:, (c - 2) * 1024:(c - 1) * 1024]
                            nc.scalar.activation(dst, sp[:, :], AF.Exp,
                                                 bias=negshift[:, :], scale=1.0)
                            # software-pipeline: previous iteration's y0
                            # matmuls interleave between S chunks
                            if prev is not None and c % 2 == 1:
                                pg_nt, pfw = prev
                                for q in range(2):
                                    qsl = slice((c // 2) * 1024 + q * 512,
                                                (c // 2) * 1024 + (q + 1) * 512)
                                    nc.tensor.matmul(
                                        y0[:, qsl], pg_nt, pfw[:, qsl],
                                        start=(nt == 1), stop=False)
                        # Z = rowsum(exp): fw half via DVE 4x tensor_scalar
                        # accum, ft half via Pool reduce (SBUF-only engine)
                        za = zp.tile([128, 2], f32, tag="z", name=f"z{nt}")
                        nc.vector.tensor_scalar(fw[:, :], fw[:, :], 0.0, None,
                                                op0=ALU.add, op1=ALU.add,
                                                accum_out=za[:, 0:1])
                        nc.vector.tensor_scalar(ft[:, :], ft[:, :], 0.0, None,
                                                op0=ALU.add, op1=ALU.add,
                                                accum_out=za[:, 1:2])
                        z = zp.tile([128, 1], f32, tag="zs", name=f"zs{nt}")
                        nc.vector.reduce_sum(z[:, :], za[:, :], axis=AX.X)
                        rz = zp.tile([128, 1], f32, tag="rz", name=f"rz{nt}")
                        nc.vector.reciprocal(rz[:, :], z[:, :])
                        g_nt = gts[:, nt * CI:(nt + 1) * CI]
                        nc.vector.tensor_scalar_mul(g_nt, g_nt, rz[:, :])
                        rzg = zp.tile([128, 1], f32, tag="rzg", name=f"rzg{nt}")
                        nc.vector.tensor_scalar_mul(rzg[:, :], rz[:, :], FP8G)
                        # normalized (gained) fp8 store of the second m-half
                        nc.gpsimd.tensor_scalar_mul(
                            fstore[:, nt * M1:(nt + 1) * M1], ft[:, :], rzg[:, :])
                        trailing(nt)
                        prev = (g_nt, fw)
                    # final iteration's y0 matmuls + drain
                    pg_nt, pfw = prev
                    for c in range(4):
                        qsl = slice(c * 512, (c + 1) * 512)
                        nc.tensor.matmul(y0[:, qsl], pg_nt, pfw[:, qsl],
                                         start=False, stop=True)
                    # preload sqrt table off the critical path (Identity/
                    # Square live in the sqrt set too; Exp is done)
                    nc.scalar.activation(statsg[:, 0:1], epsb[:, :], AF.Sqrt,
                                         scale=1.0)
                    nc.scalar.activation(ysb[:, 0:1024], y0[:, 0:1024],
                                         AF.Identity, scale=1.0)
                    nc.scalar.activation(ysb[:, 1024:2048], y0[:, 1024:2048],
                                         AF.Identity, scale=1.0)

                  # ---- phase 2: W conv + BN stats + y1 (fp8 DoubleRow) ----
                  with tc.tile_pool(name="psW", bufs=2, space="PSUM") as psW, \
                       tc.tile_pool(name="psY1", bufs=2, space="PSUM") as psY1, \
                       tc.tile_pool(name="s2p", bufs=1) as s2p:
                    s2scr = s2p.tile([128, 1024], bf16)
                    s2scr2 = s2p.tile([128, 1024], bf16)

                    def w_stats(cth, pc, pcol):
                        wsl = slice(cth * 128, (cth + 1) * 128)
                        wp = psW.tile([128, 1024], f32, tag="w")
                        for h in range(2):
                            sl = slice(pc * 1024 + h * 512,
                                       pc * 1024 + (h + 1) * 512)
                            nc.tensor.matmul(wp[:, h * 512:(h + 1) * 512],
                                             wwb[:, wsl], ysb[:, sl],
                                             start=True, stop=True)
                        # wy+w_b -> scratch; accum gives the S1 part
                        nc.scalar.activation(
                            s2scr[:, :], wp[:, :], AF.Identity,
                            bias=wb[:, cth:cth + 1], scale=1.0,
                            accum_out=s1c[:, cth * 4 + pcol:cth * 4 + pcol + 1])
                        # S2 part: square on DVE (2x bf16) + accum reduce
                        nc.vector.tensor_mul(s2scr2[:, :], s2scr[:, :],
                                             s2scr[:, :])
                        nc.vector.tensor_scalar(
                            s2scr[:, :], s2scr2[:, :], 0.0, None, op0=ALU.add,
                            op1=ALU.add,
                            accum_out=s2c[:, cth * 4 + pcol:cth * 4 + pcol + 1])

                    def y1_chunk(mc):
                        py = psY1.tile([128, 512], f32, tag="y1")
                        for t in range(NT):
                            nc.tensor.matmul(
                                py[:, :], g8[:, t * CI:(t + 1) * CI],
                                fstore[:, t * M1 + mc * 512:
                                       t * M1 + (mc + 1) * 512],
                                start=(t == 0), stop=(t == NT - 1))
                        nc.scalar.activation(
                            ysb[:, M0 + mc * 512:M0 + (mc + 1) * 512],
                            py[:, :], AF.Identity, scale=1.0 / FP8G)

                    y1_chunk(0)
                    y1_chunk(1)
                    w_stats(0, 0, 0)
                    w_stats(1, 0, 0)
                    y1_chunk(2)
                    w_stats(0, 1, 1)
                    w_stats(1, 1, 1)
                    y1_chunk(3)
                    w_stats(0, 2, 2)
                    w_stats(1, 2, 2)
                    w_stats(0, 3, 3)
                    w_stats(1, 3, 3)
                    for cth in range(2):
                        nc.vector.reduce_sum(stats[:, cth:cth + 1],
                                             s1c[:, cth * 4:(cth + 1) * 4],
                                             axis=AX.X)
                        nc.vector.reduce_sum(stats[:, 2 + cth:3 + cth],
                                             s2c[:, cth * 4:(cth + 1) * 4],
                                             axis=AX.X)
                    if dbg:
                        nc.sync.dma_start(dbg["theta"][:, :], theta[:, :])
                        nc.sync.dma_start(dbg["phi"][:, :], phi[:, :])
                        nc.sync.dma_start(dbg["gts"][:, :], gts[:, :])
                        nc.sync.dma_start(dbg["g8"][:, :], g8[:, :])
                        nc.sync.dma_start(dbg["fstore"][:, :], fstore[:, :])
                        nc.sync.dma_start(dbg["ysb"][:, :], ysb[:, :])
                        nc.sync.dma_start(dbg["stats"][:, :], stats[:, :])

                for fr in reversed(free_xA):
                    fr()
                for fr in reversed(free_xB):
                    fr()

                # ---- all-reduce + finalize ----
                nc.sync.dma_start(cc_in[:, :], stats[:, :])
                if no_collective:
                    nc.sync.dma_start(cc_out[:, :], cc_in[:, :])
                else:
                    nc.gpsimd.collective_compute(
                        "AllReduce", mybir.AluOpType.add,
                        replica_groups=[list(range(n_cores))],
                        ins=[cc_in.opt()], outs=[cc_out.opt()])
                nc.sync.dma_start(statsg[:, :], cc_out[:, :])

                with tc.tile_pool(name="fin", bufs=1) as fp2, \
                     tc.tile_pool(name="obuf", bufs=3) as obp, \
                     tc.tile_pool(name="psF", bufs=2, space="PSUM") as psF:
                    inv = 1.0 / (B * N)
                    mean2 = fp2.tile([128, 2], f32)
                    e2 = fp2.tile([128, 2], f32)
                    var2 = fp2.tile([128, 2], f32)
                    sq = fp2.tile([128, 2], f32)
                    rstd = fp2.tile([128, 2], f32)
                    acol = fp2.tile([128, 2], f32)
                    btot = fp2.tile([128, 2], f32)
                    # keep PE at full p-state through the AR wait
                    for w in range(8):
                        nc.tensor.matmul(psF.tile([128, 1024], f32, tag="f",
                                                  name=f"warm{w}")[:, 0:512],
                                         wwb[:, 0:128], ysb[:, 0:512],
                                         start=True, stop=True)
                    nc.vector.tensor_scalar_mul(mean2[:, :], statsg[:, 0:2], inv)
                    nc.vector.tensor_scalar_mul(e2[:, :], statsg[:, 2:4], inv)
                    nc.vector.tensor_mul(var2[:, :], mean2[:, :], mean2[:, :])
                    nc.vector.tensor_sub(var2[:, :], e2[:, :], var2[:, :])
                    nc.scalar.activation(sq[:, :], var2[:, :], AF.Sqrt,
                                         bias=epsb[:, :], scale=1.0)
                    nc.vector.reciprocal(rstd[:, :], sq[:, :])
                    nc.vector.tensor_mul(acol[:, :], rstd[:, :], gam)
                    # recomputed conv excludes bias: btot = (w_b - mean)*a + beta
                    nc.vector.tensor_sub(btot[:, :], wb, mean2[:, :])
                    nc.vector.tensor_mul(btot[:, :], btot[:, :], acol[:, :])
                    nc.vector.tensor_add(btot[:, :], btot[:, :], bet)
                    # recompute W conv; normalize; + l; store
                    for cth in range(2):
                        wsl = slice(cth * 128, (cth + 1) * 128)
                        lt = lb0 if cth == 0 else lb1
                        for pc in range(4):
                            psl = slice(pc * 1024, (pc + 1) * 1024)
                            fpp = psF.tile([128, 1024], f32, tag="f")
                            for h in range(2):
                                sl = slice(pc * 1024 + h * 512,
                                           pc * 1024 + (h + 1) * 512)
                                nc.tensor.matmul(fpp[:, h * 512:(h + 1) * 512],
                                                 wwb[:, wsl], ysb[:, sl],
                                                 start=True, stop=True)
                            ob = obp.tile([128, 1024], bf16, tag="ob")
                            nc.scalar.activation(ob[:, :], fpp[:, :], AF.Identity,
                                                 bias=btot[:, cth:cth + 1],
                                                 scale=acol[:, cth:cth + 1])
                            o2 = obp.tile([128, 1024], f32, tag="o2")
                            if pc % 2 == 0:
                                nc.vector.tensor_add(o2[:, :], ob[:, :],
                                                     lt[:, psl])
                                nc.sync.dma_start(out[wsl, psl], o2[:, :])
                            else:
                                nc.gpsimd.tensor_add(o2[:, :], ob[:, :],
                                                     lt[:, psl])
                                nc.scalar.dma_start(out[wsl, psl], o2[:, :])

    nc.compile()
    return nc


def _get_nc(n_cores: int):
    if n_cores not in _CACHE:
        _CACHE[n_cores] = _build(n_cores)
    return _CACHE[n_cores]


def make_in_maps(inputs: dict, n_cores: int = N_CORES):
    """Build per-core input maps from full-size inputs."""
    f = np.float32
    x = np.ascontiguousarray(inputs["x"], f).reshape(B, CS, N)
    l = np.ascontiguousarray(inputs["l"], f).reshape(B, CT, N)
    pk = np.zeros((128, PP), f)
    thwT = np.asarray(inputs["theta_w"], f).T   # [CS, CI]
    phwT = np.asarray(inputs["phi_w"], f).T
    gwT = np.asarray(inputs["g_w"], f).T
    wwT = np.asarray(inputs["w_w"], f).T        # [CI, CT]
    pk[:, _THW:_THW + 128] = thwT[0:128]
    pk[:, _THW + 128:_THW + 256] = thwT[128:256]
    pk[:, _PHW:_PHW + 128] = phwT[0:128]
    pk[:, _PHW + 128:_PHW + 256] = phwT[128:256]
    pk[:, _GW:_GW + 128] = gwT[0:128]
    pk[:, _GW + 128:_GW + 256] = gwT[128:256]
    pk[:, _WW:_WW + 256] = wwT
    pk[:, _THB] = np.asarray(inputs["theta_b"], f)
    pk[:, _WB] = np.asarray(inputs["w_b"], f)[0:128]
    pk[:, _WB + 1] = np.asarray(inputs["w_b"], f)[128:256]
    pk[:, _GAM] = np.asarray(inputs["bn_gamma"], f)[0:128]
    pk[:, _GAM + 1] = np.asarray(inputs["bn_gamma"], f)[128:256]
    pk[:, _BET] = np.asarray(inputs["bn_beta"], f)[0:128]
    pk[:, _BET + 1] = np.asarray(inputs["bn_beta"], f)[128:256]
    shared = {
        "pk": pk,
        "g_b_row": np.ascontiguousarray(inputs["g_b"], f).reshape(1, CI),
    }
    return [{"x": x[i], "lres": l[i], **shared} for i in range(n_cores)]


def kernel(**inputs) -> np.ndarray:
    from concourse import bass_utils

    nc = _get_nc(N_CORES)
    in_maps = make_in_maps(inputs, N_CORES)
    res = bass_utils.run_bass_kernel_spmd(
        nc, in_maps, core_ids=list(range(N_CORES)))
    outs = [res.results[i]["out"] for i in range(N_CORES)]
    return np.stack(outs, 0).reshape(B, CT, 64, 64).astype(np.float32)


if __name__ == "__main__":
    nc = _get_nc(N_CORES)
    print("build+compile OK")


# revision 45
# speedup vs baseline: 1.1041x; 1.1041x over previous
"""Trainium2 Bass kernel for CrossNonLocalBlock (v4).

Shapes (hardcoded): B=8, Cs=Ct=256, Ci=128, H=W=64 (N=4096 spatial).
Sharding: data-parallel over batch (1 batch element per NeuronCore, 8 cores);
1x1-conv / BN params replicated; BN batch statistics all-reduced in-kernel.

Per-core dataflow, engine-balanced around the ACT-bound softmax exp
(4096x4096 logits -> 16.8M exp/core = ~110us of ACT at 1.2GHz):

  DMA (serial ~330GB/s fabric, strict order): one packed param tensor ->
    x[:, 0:512] -> l (full) -> x rest -> xB.  The loop is gated only on
    phi (l) + the first theta/g tiles (x quarter 0).
  head-1: theta chunk 0, gT tiles 0-3 (batched 4-per-PSUM-slot), phi.
    theta bias fused into copies; phi bias dropped (constant-in-m logit
    terms cancel in softmax).
  loop over 32 n-tiles; remaining theta chunks / gT tile groups trail
  inside iters 0..9 (copies on DVE/Pool, never ACT; PSUM borrowed from
  the S-staging pool):
    S_c = theta_nt^T phi_c          (PE->PSUM fp32r, 4x[128,1024])
    f_c = exp(S_c - SHIFT)          (ACT->SBUF bf16, no accum_out)
    Z   = rowsum(f) via DVE tensor_scalar+accum (4x perf mode)
    g'_nt = gT_nt / Z               (DVE, in-place bf16)
    fstore_nt = f[:,2048:] / Z      (Pool, normalized fp8e4)
    y0[:,0:2048] += g'_nt^T f       (PE, PSUM-resident, software-pipelined
                                     one iteration behind)
  phase2: y1 = sum_nt g8_nt^T fstore_nt (PE fp8 DoubleRow, 0.5cyc/row)
    interleaved with wy chunks: PE conv + ACT Identity(bias=w_b,
    accum_out->S1), S2 via DVE tensor_tensor_reduce chains;
    AllReduce([S1|S2]).
  tail: recompute wy, out = (wy-mean)*rstd*gamma+beta + l (l resident),
    pipelined ACT/DVE/Pool/DMA.
"""

import os
import sys

import numpy as np

if "/opt/trn_rl_repo" not in sys.path:
    sys.path.insert(0, "/opt/trn_rl_repo")

B, CS, CT, CI, N = 8, 256, 256, 128, 4096
NT = N // 128          # 32 n-tiles
M0 = 2048              # m-columns accumulated in PSUM during the n-loop
M1 = N - M0            # m-columns stored normalized fp8 for the 2nd pass
SHIFT = 50.0           # global logit shift fed to exp() as ACT bias
FP8G = 128.0           # fstore gain: keeps f*G/Z in fp8e4's normal range
BN_EPS = 1e-5
N_CORES = 8

# packed param layout (columns of the [128, PP] tensor)
_THW, _PHW, _GW, _WW = 0, 256, 512, 768
_THB, _WB, _GAM, _BET = 1024, 1025, 1027, 1029
PP = 1031

_CACHE = {}


def _build(n_cores: int, no_collective: bool = False):
    import concourse.bass as bass
    import concourse.mybir as mybir
    import concourse.tile as tile
    from concourse import bacc

    f32 = mybir.dt.float32
    f32r = mybir.dt.float32r
    bf16 = mybir.dt.bfloat16
    f8 = mybir.dt.float8e4
    f16 = mybir.dt.float16
    AF = mybir.ActivationFunctionType
    AX = mybir.AxisListType
    ALU = mybir.AluOpType
    DR = mybir.MatmulPerfMode.DoubleRow

    nc = bacc.Bacc("TRN2", target_bir_lowering=False, debug=False,
                   num_devices=n_cores)

    # ---- DRAM I/O (per-core) ----
    x = nc.dram_tensor("x", [CS, N], f32, kind="ExternalInput").ap()
    lres = nc.dram_tensor("lres", [CT, N], f32, kind="ExternalInput").ap()
    pk_d = nc.dram_tensor("pk", [128, PP], f32, kind="ExternalInput").ap()
    gb_d = nc.dram_tensor("g_b_row", [1, CI], f32, kind="ExternalInput").ap()
    out = nc.dram_tensor("out", [CT, N], f32, kind="ExternalOutput").ap()
    dbg = {}
    if os.environ.get("KDBG"):
        dbg["theta"] = nc.dram_tensor("d_theta", [CI, N], bf16,
                                      kind="ExternalOutput").ap()
        dbg["phi"] = nc.dram_tensor("d_phi", [CI, N], bf16,
                                    kind="ExternalOutput").ap()
        dbg["gts"] = nc.dram_tensor("d_gts", [128, NT * CI], bf16,
                                    kind="ExternalOutput").ap()
        dbg["g8"] = nc.dram_tensor("d_g8", [128, NT * CI], f8,
                                   kind="ExternalOutput").ap()
        dbg["fstore"] = nc.dram_tensor("d_fstore", [128, NT * M1], f8,
                                       kind="ExternalOutput").ap()
        dbg["ysb"] = nc.dram_tensor("d_ysb", [CI, N], bf16,
                                    kind="ExternalOutput").ap()
        dbg["stats"] = nc.dram_tensor("d_stats", [128, 4], f32,
                                      kind="ExternalOutput").ap()

    def r(ap):
        return ap.bitcast(f32r)

    with tile.TileContext(nc) as tc:
        with tc.tile_pool(name="persist", bufs=1) as pp:
            theta = pp.tile([CI, N], f32r)       # 16KB/part (rounded f32)
            phi = pp.tile([CI, N], f32r)         # 16KB/part
            gts = pp.tile([128, NT * CI], bf16)  # gT tiles (later scaled g') 8KB
            g8 = pp.tile([128, NT * CI], f8)     # unscaled gT in fp8       4KB
            lb0 = pp.tile([128, N], bf16)        # l residual, bf16         8KB
            lb1 = pp.tile([128, N], bf16)        # 8KB
            ysb = pp.tile([CI, N], bf16)         # attention out yT         8KB
            pk = pp.tile([128, PP], f32)         # packed params            4KB
            gwb = pp.tile([128, 2 * CI], f16)
            thwb = pp.tile([128, 2 * CI], f16)
            phwr = pp.tile([128, 2 * CI], f32r)
            wwb = pp.tile([CI, CT], bf16)
            gbr = pp.tile([1, CI], f32)
            gbrb = pp.tile([1, CI], f16)
            onesb = pp.tile([1, 128], f16)
            negshift = pp.tile([128, 1], f32)
            epsb = pp.tile([128, 1], f32)
            stats = pp.tile([128, 4], f32)       # [S1h0 S1h1 S2h0 S2h1]
            statsg = pp.tile([128, 4], f32)
            s1c = pp.tile([128, 8], f32)
            s2c = pp.tile([128, 8], f32)

            thw = pk[:, _THW:_THW + 256]
            phw = pk[:, _PHW:_PHW + 256]
            gw = pk[:, _GW:_GW + 256]
            ww = pk[:, _WW:_WW + 256]
            thb = pk[:, _THB:_THB + 1]
            wb = pk[:, _WB:_WB + 2]
            gam = pk[:, _GAM:_GAM + 2]
            bet = pk[:, _BET:_BET + 2]

            nc.sync.dma_start(pk[:, :], pk_d[:, :])
            nc.sync.dma_start(gbr[:, :], gb_d[:, :])
            nc.vector.memset(onesb[:, :], 1.0)
            nc.vector.memset(negshift[:, :], -SHIFT)
            nc.vector.memset(epsb[:, :], BN_EPS)
            # preload the exp ACT table while the head DMAs stream
            nc.scalar.activation(stats[:, 0:1], negshift[:, :], AF.Exp,
                                 scale=1.0)
            nc.vector.tensor_copy(gwb[:, :], gw)
            nc.vector.tensor_copy(thwb[:, :], thw)
            nc.vector.tensor_copy(phwr[:, :], phw)
            nc.vector.tensor_copy(gbrb[:, :], gbr[:, :])
            nc.vector.tensor_copy(wwb[:, :], ww)

            with tc.tile_pool(name="dram", bufs=1, space="DRAM") as dp:
                cc_in = dp.tile([128, 4], f32)
                cc_out = dp.tile([128, 4], f32,
                                 addr_space="Shared" if n_cores > 1 else "Local")

                # bulk loads: x quarter0 -> l -> x rest -> xB
                xB, xb16B, free_xB = [], [], []
                xA, xb16A, free_xA = [], [], []
                for k in range(2):
                    t, fr = tc.tile([128, M1], f32, name=f"xB{k}")
                    xB.append(t)
                    free_xB.append(fr)
                    t, fr = tc.tile([128, M1], f16, name=f"xb16B{k}")
                    xb16B.append(t)
                    free_xB.append(fr)
                for k in range(2):
                    t, fr = tc.tile([128, M0], f32, name=f"xA{k}")
                    xA.append(t)
                    free_xA.append(fr)
                    t, fr = tc.tile([128, M0], f16, name=f"xb16A{k}")
                    xb16A.append(t)
                    free_xA.append(fr)
                l0, free_l0 = tc.tile([128, N], f32r, name="l0")
                l1, free_l1 = tc.tile([128, N], f32r, name="l1")
                lt0, free_lt0 = tc.tile([128, N], f32, name="lt0")
                lt1, free_lt1 = tc.tile([128, N], f32, name="lt1")
                nc.sync.dma_start(lt0[:, :], lres[0:128, :])
                for k in range(2):
                    nc.sync.dma_start(xA[k][:, 0:512], x[k * 128:(k + 1) * 128, 0:512])
                nc.sync.dma_start(lt1[:, :], lres[128:256, :])
                for k in range(2):
                    nc.sync.dma_start(xA[k][:, 512:M0],
                                      x[k * 128:(k + 1) * 128, 512:M0])
                for k in range(2):
                    nc.sync.dma_start(xB[k][:, :], x[k * 128:(k + 1) * 128, M0:N])
                # round l to f32r (phi's matmul operand) in the DMA shadow;
                # l1 split ACT/DVE so phi-k1 starts right behind it
                for c in range(4):
                    sl = slice(c * 1024, (c + 1) * 1024)
                    nc.scalar.activation(l0[:, sl], lt0[:, sl], AF.Identity,
                                         scale=1.0)
                nc.vector.tensor_copy(lb0[:, :], lt0[:, :])
                for c in range(4):
                    sl = slice(c * 1024, (c + 1) * 1024)
                    if c % 2 == 0:
                        nc.scalar.activation(l1[:, sl], lt1[:, sl], AF.Identity,
                                             scale=1.0)
                    else:
                        nc.vector.tensor_copy(l1[:, sl], lt1[:, sl])

                def theta_chunk(ps, c, xk, base, on_act):
                    pt = ps.tile([128, 1024], f32, tag="s", name=f"thps{base + c}")
                    sl = slice(base + c * 512, base + (c + 1) * 512)
                    for k in range(2):
                        nc.tensor.matmul(pt[:, 0:512],
                                         thwb[:, k * CI:(k + 1) * CI],
                                         xk[k][:, c * 512:(c + 1) * 512],
                                         start=(k == 0), stop=(k == 1))
                    if on_act:
                        nc.scalar.activation(theta[:, sl], pt[:, 0:512],
                                             AF.Identity, bias=thb, scale=1.0)
                    else:
                        nc.vector.tensor_scalar(theta[:, sl], pt[:, 0:512],
                                                thb, None, op0=ALU.add)

                def g_group(ps, grp, xk, on_act):
                    # 4 gT tiles batched into one PSUM slot, two wide copies
                    pg = ps.tile([128, 1024], f32, tag="s", name=f"gps{grp}")
                    for j in range(4):
                        nt = grp * 4 + j
                        nsl = slice((nt * 128) % M0, (nt * 128) % M0 + 128)
                        osl = slice(j * 128, (j + 1) * 128)
                        nc.tensor.matmul(pg[:, osl], xk[0][:, nsl], gwb[:, 0:CI],
                                         start=True, stop=False)
                        nc.tensor.matmul(pg[:, osl], xk[1][:, nsl],
                                         gwb[:, CI:2 * CI], start=False, stop=False)
                        nc.tensor.matmul(pg[:, osl], onesb[:, :], gbrb[:, :],
                                         start=False, stop=True)
                    gsl = slice(grp * 512, (grp + 1) * 512)
                    if on_act:
                        nc.scalar.activation(gts[:, gsl], pg[:, 0:512],
                                             AF.Identity, scale=1.0)
                        nc.scalar.activation(g8[:, gsl], gts[:, gsl],
                                             AF.Identity, scale=1.0)
                    else:
                        nc.vector.tensor_copy(gts[:, gsl], pg[:, 0:512])

                # ---- head-1 (own PSUM pool, closed before the loop) ----
                with tc.tile_pool(name="ps0", bufs=4, space="PSUM") as ps0:
                    for k in range(2):
                        nc.vector.tensor_copy(xb16A[k][:, 0:512], xA[k][:, 0:512])
                    theta_chunk(ps0, 0, xb16A, 0, True)
                    g_group(ps0, 0, xb16A, True)
                    # phi: ALL k0 partials first (run in l1's DMA shadow,
                    # re-emitted once as a PE p-state bridge), then all k1.
                    # Copies split ACT/DVE to shorten the loop-gating drain.
                    pts = []
                    for c in range(4):
                        pts.append(ps0.tile([128, 1024], f32, tag="s",
                                            name=f"phps{c}"))
                    # p-state bridge: complete throwaway groups first
                    for rep in range(2):
                        for c in range(4):
                            if rep == 1 and c >= 2:
                                continue
                            for h in range(2):
                                sl = slice(c * 1024 + h * 512,
                                           c * 1024 + (h + 1) * 512)
                                nc.tensor.matmul(
                                    pts[c][:, h * 512:(h + 1) * 512],
                                    phwr[:, 0:CI], l0[:, sl],
                                    start=True, stop=True)
                    for c in range(4):
                        for h in range(2):
                            sl = slice(c * 1024 + h * 512,
                                       c * 1024 + (h + 1) * 512)
                            nc.tensor.matmul(
                                pts[c][:, h * 512:(h + 1) * 512],
                                phwr[:, 0:CI], l0[:, sl],
                                start=True, stop=False)
                    nc.vector.tensor_copy(lb1[:, :], lt1[:, :])
                    for c in range(4):
                        for h in range(2):
                            sl = slice(c * 1024 + h * 512, c * 1024 + (h + 1) * 512)
                            nc.tensor.matmul(
                                pts[c][:, h * 512:(h + 1) * 512],
                                phwr[:, CI:2 * CI], l1[:, sl],
                                start=False, stop=True)
                        psl = slice(c * 1024, (c + 1) * 1024)
                        if c % 2 == 0:
                            nc.scalar.activation(phi[:, psl], pts[c][:, :],
                                                 AF.Identity, scale=1.0)
                        else:
                            nc.vector.tensor_copy(phi[:, psl], pts[c][:, :])
                free_lt1()
                free_lt0()
                free_l1()
                free_l0()

                # ---- attention n-loop (fstore outlives the loop pools) ----
                with tc.tile_pool(name="fstore", bufs=1) as fsp:
                  fstore = fsp.tile([128, NT * M1], f8)   # 64KB/part
                  with tc.tile_pool(name="psS", bufs=2, space="PSUM") as psS, \
                       tc.tile_pool(name="psY0", bufs=1, space="PSUM") as psY0, \
                       tc.tile_pool(name="zp", bufs=2) as zp, \
                       tc.tile_pool(name="fwp", bufs=2) as fwp, \
                       tc.tile_pool(name="ftp", bufs=2) as ftp:
                    y0 = psY0.tile([CI, M0], f32)
                    prev = None  # (g_nt ap, fw tile) pending y0 matmuls

                    def pre_trailing(nt):
                        # pure-DVE feed copies (no PSUM involvement)
                        if nt in (0, 1, 2):   # xb16A rest: 2x[128,512] per iter
                            for k in range(2):
                                sl = slice(512 + nt * 512, 1024 + nt * 512)
                                nc.gpsimd.tensor_copy(xb16A[k][:, sl],
                                                      xA[k][:, sl])
                        if 3 <= nt <= 6:      # xb16B k0/k1 in [128,1024] pieces
                            k, q = (nt - 3) % 2, (nt - 3) // 2
                            sl = slice(q * 1024, (q + 1) * 1024)
                            nc.gpsimd.tensor_copy(xb16B[k][:, sl], xB[k][:, sl])

                    def trailing(nt):
                        # ONE PSUM-slot borrower per iteration, emitted at the
                        # iteration end so its slot-freeing copy has a whole
                        # iteration of slack before the slot is needed again.
                        # gT group j lands at iter 2j-1 (needed at iter 4j);
                        # theta chunk c of each half interleaves on even iters.
                        if nt % 2 == 1 and 1 <= nt <= 13:    # gT groups 1-7
                            grp = (nt + 1) // 2
                            g_group(psS, grp, xb16A if grp < 4 else xb16B, False)
                        elif nt in (2, 4, 6):                # theta-A chunks 1-3
                            theta_chunk(psS, nt // 2, xb16A, 0, False)
                        elif nt in (8, 10, 12, 14):          # theta-B chunks
                            theta_chunk(psS, (nt - 8) // 2, xb16B, M0, False)
                        if nt % 2 == 0 and 2 <= nt <= 14:
                            # fp8 copy of the group finished last iteration,
                            # before any of its tiles get scaled in-place
                            grp = nt // 2
                            gsl = slice(grp * 512, (grp + 1) * 512)
                            nc.vector.tensor_copy(g8[:, gsl], gts[:, gsl])

                    for nt in range(NT):
                        pre_trailing(nt)
                        th_nt = theta[:, nt * 128:(nt + 1) * 128]
                        fw = fwp.tile([128, M0], bf16, tag="fw", name=f"fw{nt}")
                        ft = ftp.tile([128, M1], bf16, tag="ft", name=f"ft{nt}")
                        for c in range(4):
                            sp = psS.tile([128, 1024], f32, tag="s")
                            for h in range(2):
                                sl = slice(c * 1024 + h * 512,
                                           c * 1024 + (h + 1) * 512)
                                nc.tensor.matmul(sp[:, h * 512:(h + 1) * 512],
                                                 th_nt, phi[:, sl],
                                                 start=True, stop=True)
                            if c < 2:
                                dst = fw[:, c * 1024:(c + 1) * 1024]
                            else:
                                dst = ft[:, (c - 2) * 1024:(c - 1) * 1024]
                            nc.scalar.activation(dst, sp[:, :], AF.Exp,
                                                 bias=negshift[:, :], scale=1.0)
                            # software-pipeline: previous iteration's y0
                            # matmuls interleave between S chunks
                            if prev is not None and c % 2 == 1:
                                pg_nt, pfw = prev
                                for q in range(2):
                                    qsl = slice((c // 2) * 1024 + q * 512,
                                                (c // 2) * 1024 + (q + 1) * 512)
                                    nc.tensor.matmul(
                                        y0[:, qsl], pg_nt, pfw[:, qsl],
                                        start=(nt == 1), stop=False)
                        # Z = rowsum(exp): fw half via DVE 4x tensor_scalar
                        # accum, ft half via Pool reduce (SBUF-only engine)
                        za = zp.tile([128, 2], f32, tag="z", name=f"z{nt}")
                        nc.vector.tensor_scalar(fw[:, :], fw[:, :], 0.0, None,
                                                op0=ALU.add, op1=ALU.add,
                                                accum_out=za[:, 0:1])
                        nc.vector.tensor_scalar(ft[:, :], ft[:, :], 0.0, None,
                                                op0=ALU.add, op1=ALU.add,
                                                accum_out=za[:, 1:2])
                        z = zp.tile([128, 1], f32, tag="zs", name=f"zs{nt}")
                        nc.vector.reduce_sum(z[:, :], za[:, :], axis=AX.X)
                        rz = zp.tile([128, 1], f32, tag="rz", name=f"rz{nt}")
                        nc.vector.reciprocal(rz[:, :], z[:, :])
                        g_nt = gts[:, nt * CI:(nt + 1) * CI]
                        nc.vector.tensor_scalar_mul(g_nt, g_nt, rz[:, :])
                        rzg = zp.tile([128, 1], f32, tag="rzg", name=f"rzg{nt}")
                        nc.vector.tensor_scalar_mul(rzg[:, :], rz[:, :], FP8G)
                        # normalized (gained) fp8 store of the second m-half
                        nc.gpsimd.tensor_scalar_mul(
                            fstore[:, nt * M1:(nt + 1) * M1], ft[:, :], rzg[:, :])
                        trailing(nt)
                        prev = (g_nt, fw)
                    # final iteration's y0 matmuls + drain
                    pg_nt, pfw = prev
                    for c in range(4):
                        qsl = slice(c * 512, (c + 1) * 512)
                        nc.tensor.matmul(y0[:, qsl], pg_nt, pfw[:, qsl],
                                         start=False, stop=True)
                    # preload sqrt table off the critical path (Identity/
                    # Square live in the sqrt set too; Exp is done)
                    nc.scalar.activation(statsg[:, 0:1], epsb[:, :], AF.Sqrt,
                                         scale=1.0)
                    nc.scalar.activation(ysb[:, 0:1024], y0[:, 0:1024],
                                         AF.Identity, scale=1.0)
                    nc.scalar.activation(ysb[:, 1024:2048], y0[:, 1024:2048],
                                         AF.Identity, scale=1.0)

                  # ---- phase 2: W conv + BN stats + y1 (fp8 DoubleRow) ----
                  with tc.tile_pool(name="psW", bufs=2, space="PSUM") as psW, \
                       tc.tile_pool(name="psY1", bufs=2, space="PSUM") as psY1, \
                       tc.tile_pool(name="s2p", bufs=2) as s2p:

                    def w_stats(cth, pc, pcol):
                        s2scr = s2p.tile([128, 1024], bf16, tag="sa",
                                         name=f"sa{cth}_{pc}")
                        s2scr2 = s2p.tile([128, 1024], bf16, tag="sb",
                                          name=f"sb{cth}_{pc}")
                        wsl = slice(cth * 128, (cth + 1) * 128)
                        wp = psW.tile([128, 1024], f32, tag="w")
                        for h in range(2):
                            sl = slice(pc * 1024 + h * 512,
                                       pc * 1024 + (h + 1) * 512)
                            nc.tensor.matmul(wp[:, h * 512:(h + 1) * 512],
                                             wwb[:, wsl], ysb[:, sl],
                                             start=True, stop=True)
                        # wy+w_b -> scratch; accum gives the S1 part
                        nc.scalar.activation(
                            s2scr[:, :], wp[:, :], AF.Identity,
                            bias=wb[:, cth:cth + 1], scale=1.0,
                            accum_out=s1c[:, cth * 4 + pcol:cth * 4 + pcol + 1])
                        # S2 part: square on DVE (2x bf16) + accum reduce
                        nc.vector.tensor_mul(s2scr2[:, :], s2scr[:, :],
                                             s2scr[:, :])
                        nc.vector.tensor_scalar(
                            s2scr[:, :], s2scr2[:, :], 0.0, None, op0=ALU.add,
                            op1=ALU.add,
                            accum_out=s2c[:, cth * 4 + pcol:cth * 4 + pcol + 1])

                    def y1_chunk(mc):
                        py = psY1.tile([128, 512], f32, tag="y1")
                        for t in range(NT // 2):
                            gpair = g8[:, 2 * t * CI:(2 * t + 2) * CI] \
                                .rearrange("p (two c) -> p two c", two=2)
                            fpair = fstore[:, 2 * t * M1:(2 * t + 2) * M1] \
                                .rearrange("p (two m) -> p two m", two=2)[
                                    :, :, mc * 512:(mc + 1) * 512]
                            nc.tensor.matmul(py[:, :], gpair, fpair,
                                             start=(t == 0),
                                             stop=(t == NT // 2 - 1),
                                             perf_mode=DR)
                        nc.scalar.activation(
                            ysb[:, M0 + mc * 512:M0 + (mc + 1) * 512],
                            py[:, :], AF.Identity, scale=1.0 / FP8G)

                    y1_chunk(0)
                    y1_chunk(1)
                    w_stats(0, 0, 0)
                    w_stats(1, 0, 0)
                    y1_chunk(2)
                    w_stats(0, 1, 1)
                    w_stats(1, 1, 1)
                    y1_chunk(3)
                    w_stats(0, 2, 2)
                    w_stats(1, 2, 2)
                    w_stats(0, 3, 3)
                    w_stats(1, 3, 3)
                    for cth in range(2):
                        nc.vector.reduce_sum(stats[:, cth:cth + 1],
                                             s1c[:, cth * 4:(cth + 1) * 4],
                                             axis=AX.X)
                        nc.vector.reduce_sum(stats[:, 2 + cth:3 + cth],
                                             s2c[:, cth * 4:(cth + 1) * 4],
                                             axis=AX.X)
                    if dbg:
                        nc.sync.dma_start(dbg["theta"][:, :], theta[:, :])
                        nc.sync.dma_start(dbg["phi"][:, :], phi[:, :])
                        nc.sync.dma_start(dbg["gts"][:, :], gts[:, :])
                        nc.sync.dma_start(dbg["g8"][:, :], g8[:, :])
                        nc.sync.dma_start(dbg["fstore"][:, :], fstore[:, :])
                        nc.sync.dma_start(dbg["ysb"][:, :], ysb[:, :])
                        nc.sync.dma_start(dbg["stats"][:, :], stats[:, :])

                for fr in reversed(free_xA):
                    fr()
                for fr in reversed(free_xB):
                    fr()

                # ---- all-reduce + finalize ----
                nc.sync.dma_start(cc_in[:, :], stats[:, :])
                if no_collective:
                    nc.sync.dma_start(cc_out[:, :], cc_in[:, :])
                else:
                    nc.gpsimd.collective_compute(
                        "AllReduce", mybir.AluOpType.add,
                        replica_groups=[list(range(n_cores))],
                        ins=[cc_in.opt()], outs=[cc_out.opt()])
                nc.sync.dma_start(statsg[:, :], cc_out[:, :])

                with tc.tile_pool(name="fin", bufs=1) as fp2, \
                     tc.tile_pool(name="obuf", bufs=3) as obp, \
                     tc.tile_pool(name="psF", bufs=2, space="PSUM") as psF:
                    inv = 1.0 / (B * N)
                    mean2 = fp2.tile([128, 2], f32)
                    e2 = fp2.tile([128, 2], f32)
                    var2 = fp2.tile([128, 2], f32)
                    sq = fp2.tile([128, 2], f32)
                    rstd = fp2.tile([128, 2], f32)
                    acol = fp2.tile([128, 2], f32)
                    btot = fp2.tile([128, 2], f32)
                    # keep PE at full p-state through the AR wait
                    for w in range(8):
                        nc.tensor.matmul(psF.tile([128, 1024], f32, tag="f",
                                                  name=f"warm{w}")[:, 0:512],
                                         wwb[:, 0:128], ysb[:, 0:512],
                                         start=True, stop=True)
                    nc.vector.tensor_scalar_mul(mean2[:, :], statsg[:, 0:2], inv)
                    nc.vector.tensor_scalar_mul(e2[:, :], statsg[:, 2:4], inv)
                    nc.vector.tensor_mul(var2[:, :], mean2[:, :], mean2[:, :])
                    nc.vector.tensor_sub(var2[:, :], e2[:, :], var2[:, :])
                    nc.scalar.activation(sq[:, :], var2[:, :], AF.Sqrt,
                                         bias=epsb[:, :], scale=1.0)
                    nc.vector.reciprocal(rstd[:, :], sq[:, :])
                    nc.vector.tensor_mul(acol[:, :], rstd[:, :], gam)
                    # recomputed conv excludes bias: btot = (w_b - mean)*a + beta
                    nc.vector.tensor_sub(btot[:, :], wb, mean2[:, :])
                    nc.vector.tensor_mul(btot[:, :], btot[:, :], acol[:, :])
                    nc.vector.tensor_add(btot[:, :], btot[:, :], bet)
                    # recompute W conv; normalize; + l; store
                    for cth in range(2):
                        wsl = slice(cth * 128, (cth + 1) * 128)
                        lt = lb0 if cth == 0 else lb1
                        for pc in range(4):
                            psl = slice(pc * 1024, (pc + 1) * 1024)
                            fpp = psF.tile([128, 1024], f32, tag="f")
                            for h in range(2):
                                sl = slice(pc * 1024 + h * 512,
                                           pc * 1024 + (h + 1) * 512)
                                nc.tensor.matmul(fpp[:, h * 512:(h + 1) * 512],
                                                 wwb[:, wsl], ysb[:, sl],
                                                 start=True, stop=True)
                            ob = obp.tile([128, 1024], bf16, tag="ob")
                            nc.scalar.activation(ob[:, :], fpp[:, :], AF.Identity,
                                                 bias=btot[:, cth:cth + 1],
                                                 scale=acol[:, cth:cth + 1])
                            o2 = obp.tile([128, 1024], f32, tag="o2")
                            if pc % 2 == 0:
                                nc.vector.tensor_add(o2[:, :], ob[:, :],
                                                     lt[:, psl])
                                nc.sync.dma_start(out[wsl, psl], o2[:, :])
                            else:
                                nc.gpsimd.tensor_add(o2[:, :], ob[:, :],
                                                     lt[:, psl])
                                nc.scalar.dma_start(out[wsl, psl], o2[:, :])

    nc.compile()
    return nc


def _get_nc(n_cores: int):
    if n_cores not in _CACHE:
        _CACHE[n_cores] = _build(n_cores)
    return _CACHE[n_cores]


def make_in_maps(inputs: dict, n_cores: int = N_CORES):
    """Build per-core input maps from full-size inputs."""
    f = np.float32
    x = np.ascontiguousarray(inputs["x"], f).reshape(B, CS, N)
    l = np.ascontiguousarray(inputs["l"], f).reshape(B, CT, N)
    pk = np.zeros((128, PP), f)
    thwT = np.asarray(inputs["theta_w"], f).T   # [CS, CI]
    phwT = np.asarray(inputs["phi_w"], f).T
    gwT = np.asarray(inputs["g_w"], f).T
    wwT = np.asarray(inputs["w_w"], f).T        # [CI, CT]
    pk[:, _THW:_THW + 128] = thwT[0:128]
    pk[:, _THW + 128:_THW + 256] = thwT[128:256]
    pk[:, _PHW:_PHW + 128] = phwT[0:128]
    pk[:, _PHW + 128:_PHW + 256] = phwT[128:256]
    pk[:, _GW:_GW + 128] = gwT[0:128]
    pk[:, _GW + 128:_GW + 256] = gwT[128:256]
    pk[:, _WW:_WW + 256] = wwT
    pk[:, _THB] = np.asarray(inputs["theta_b"], f)
    pk[:, _WB] = np.asarray(inputs["w_b"], f)[0:128]
    pk[:, _WB + 1] = np.asarray(inputs["w_b"], f)[128:256]
    pk[:, _GAM] = np.asarray(inputs["bn_gamma"], f)[0:128]
    pk[:, _GAM + 1] = np.asarray(inputs["bn_gamma"], f)[128:256]
    pk[:, _BET] = np.asarray(inputs["bn_beta"], f)[0:128]
    pk[:, _BET + 1] = np.asarray(inputs["bn_beta"], f)[128:256]
    shared = {
        "pk": pk,
        "g_b_row": np.ascontiguousarray(inputs["g_b"], f).reshape(1, CI),
    }
    return [{"x": x[i], "lres": l[i], **shared} for i in range(n_cores)]


def kernel(**inputs) -> np.ndarray:
    from concourse import bass_utils

    nc = _get_nc(N_CORES)
    in_maps = make_in_maps(inputs, N_CORES)
    res = bass_utils.run_bass_kernel_spmd(
        nc, in_maps, core_ids=list(range(N_CORES)))
    outs = [res.results[i]["out"] for i in range(N_CORES)]
    return np.stack(outs, 0).reshape(B, CT, 64, 64).astype(np.float32)


if __name__ == "__main__":
    nc = _get_nc(N_CORES)
    print("build+compile OK")


# revision 47
# speedup vs baseline: 1.1095x; 1.0049x over previous
"""Trainium2 Bass kernel for CrossNonLocalBlock (v4).

Shapes (hardcoded): B=8, Cs=Ct=256, Ci=128, H=W=64 (N=4096 spatial).
Sharding: data-parallel over batch (1 batch element per NeuronCore, 8 cores);
1x1-conv / BN params replicated; BN batch statistics all-reduced in-kernel.

Per-core dataflow, engine-balanced around the ACT-bound softmax exp
(4096x4096 logits -> 16.8M exp/core = ~110us of ACT at 1.2GHz):

  DMA (serial ~330GB/s fabric, strict order): one packed param tensor ->
    x[:, 0:512] -> l (full) -> x rest -> xB.  The loop is gated only on
    phi (l) + the first theta/g tiles (x quarter 0).
  head-1: theta chunk 0, gT tiles 0-3 (batched 4-per-PSUM-slot), phi.
    theta bias fused into copies; phi bias dropped (constant-in-m logit
    terms cancel in softmax).
  loop over 32 n-tiles; remaining theta chunks / gT tile groups trail
  inside iters 0..9 (copies on DVE/Pool, never ACT; PSUM borrowed from
  the S-staging pool):
    S_c = theta_nt^T phi_c          (PE->PSUM fp32r, 4x[128,1024])
    f_c = exp(S_c - SHIFT)          (ACT->SBUF bf16, no accum_out)
    Z   = rowsum(f) via DVE tensor_scalar+accum (4x perf mode)
    g'_nt = gT_nt / Z               (DVE, in-place bf16)
    fstore_nt = f[:,2048:] / Z      (Pool, normalized fp8e4)
    y0[:,0:2048] += g'_nt^T f       (PE, PSUM-resident, software-pipelined
                                     one iteration behind)
  phase2: y1 = sum_nt g8_nt^T fstore_nt (PE fp8 DoubleRow, 0.5cyc/row)
    interleaved with wy chunks: PE conv + ACT Identity(bias=w_b,
    accum_out->S1), S2 via DVE tensor_tensor_reduce chains;
    AllReduce([S1|S2]).
  tail: recompute wy, out = (wy-mean)*rstd*gamma+beta + l (l resident),
    pipelined ACT/DVE/Pool/DMA.
"""

import os
import sys

import numpy as np

if "/opt/trn_rl_repo" not in sys.path:
    sys.path.insert(0, "/opt/trn_rl_repo")

B, CS, CT, CI, N = 8, 256, 256, 128, 4096
NT = N // 128          # 32 n-tiles
M0 = 2048              # m-columns accumulated in PSUM during the n-loop
M1 = N - M0            # m-columns stored normalized fp8 for the 2nd pass
SHIFT = 50.0           # global logit shift fed to exp() as ACT bias
FP8G = 128.0           # fstore gain: keeps f*G/Z in fp8e4's normal range
BN_EPS = 1e-5
N_CORES = 8

# packed param layout (columns of the [128, PP] tensor)
_THW, _PHW, _GW, _WW = 0, 256, 512, 768
_THB, _WB, _GAM, _BET = 1024, 1025, 1027, 1029
PP = 1031

_CACHE = {}


def _build(n_cores: int, no_collective: bool = False):
    import concourse.bass as bass
    import concourse.mybir as mybir
    import concourse.tile as tile
    from concourse import bacc

    f32 = mybir.dt.float32
    f32r = mybir.dt.float32r
    bf16 = mybir.dt.bfloat16
    f8 = mybir.dt.float8e4
    f16 = mybir.dt.float16
    AF = mybir.ActivationFunctionType
    AX = mybir.AxisListType
    ALU = mybir.AluOpType
    DR = mybir.MatmulPerfMode.DoubleRow

    nc = bacc.Bacc("TRN2", target_bir_lowering=False, debug=False,
                   num_devices=n_cores)

    # ---- DRAM I/O (per-core) ----
    x = nc.dram_tensor("x", [CS, N], f32, kind="ExternalInput").ap()
    lres = nc.dram_tensor("lres", [CT, N], f32, kind="ExternalInput").ap()
    pk_d = nc.dram_tensor("pk", [128, PP], f32, kind="ExternalInput").ap()
    gb_d = nc.dram_tensor("g_b_row", [1, CI], f32, kind="ExternalInput").ap()
    out = nc.dram_tensor("out", [CT, N], f32, kind="ExternalOutput").ap()
    dbg = {}
    if os.environ.get("KDBG"):
        dbg["theta"] = nc.dram_tensor("d_theta", [CI, N], bf16,
                                      kind="ExternalOutput").ap()
        dbg["phi"] = nc.dram_tensor("d_phi", [CI, N], bf16,
                                    kind="ExternalOutput").ap()
        dbg["gts"] = nc.dram_tensor("d_gts", [128, NT * CI], bf16,
                                    kind="ExternalOutput").ap()
        dbg["g8"] = nc.dram_tensor("d_g8", [128, NT * CI], f8,
                                   kind="ExternalOutput").ap()
        dbg["fstore"] = nc.dram_tensor("d_fstore", [128, NT * M1], f8,
                                       kind="ExternalOutput").ap()
        dbg["ysb"] = nc.dram_tensor("d_ysb", [CI, N], bf16,
                                    kind="ExternalOutput").ap()
        dbg["stats"] = nc.dram_tensor("d_stats", [128, 4], f32,
                                      kind="ExternalOutput").ap()

    def r(ap):
        return ap.bitcast(f32r)

    with tile.TileContext(nc) as tc:
        with tc.tile_pool(name="persist", bufs=1) as pp:
            theta = pp.tile([CI, N], f32r)       # 16KB/part (rounded f32)
            phi = pp.tile([CI, N], f32r)         # 16KB/part
            gts = pp.tile([128, NT * CI], bf16)  # gT tiles (later scaled g') 8KB
            g8 = pp.tile([128, NT * CI], f8)     # unscaled gT in fp8       4KB
            lb0 = pp.tile([128, N], bf16)        # l residual, bf16         8KB
            lb1 = pp.tile([128, N], bf16)        # 8KB
            ysb = pp.tile([CI, N], bf16)         # attention out yT         8KB
            pk = pp.tile([128, PP], f32)         # packed params            4KB
            gwb = pp.tile([128, 2 * CI], f16)
            thwb = pp.tile([128, 2 * CI], f16)
            phwr = pp.tile([128, 2 * CI], f32r)
            wwb = pp.tile([CI, CT], bf16)
            gbr = pp.tile([1, CI], f32)
            gbrb = pp.tile([1, CI], f16)
            onesb = pp.tile([1, 128], f16)
            negshift = pp.tile([128, 1], f32)
            epsb = pp.tile([128, 1], f32)
            stats = pp.tile([128, 4], f32)       # [S1h0 S1h1 S2h0 S2h1]
            statsg = pp.tile([128, 4], f32)
            s1c = pp.tile([128, 8], f32)
            s2c = pp.tile([128, 8], f32)

            thw = pk[:, _THW:_THW + 256]
            phw = pk[:, _PHW:_PHW + 256]
            gw = pk[:, _GW:_GW + 256]
            ww = pk[:, _WW:_WW + 256]
            thb = pk[:, _THB:_THB + 1]
            wb = pk[:, _WB:_WB + 2]
            gam = pk[:, _GAM:_GAM + 2]
            bet = pk[:, _BET:_BET + 2]

            nc.sync.dma_start(pk[:, :], pk_d[:, :])
            nc.sync.dma_start(gbr[:, :], gb_d[:, :])
            nc.vector.memset(onesb[:, :], 1.0)
            nc.vector.memset(negshift[:, :], -SHIFT)
            nc.vector.memset(epsb[:, :], BN_EPS)
            # preload the exp ACT table while the head DMAs stream
            nc.scalar.activation(stats[:, 0:1], negshift[:, :], AF.Exp,
                                 scale=1.0)
            nc.vector.tensor_copy(gwb[:, :], gw)
            nc.vector.tensor_copy(thwb[:, :], thw)
            nc.vector.tensor_copy(phwr[:, :], phw)
            nc.vector.tensor_copy(gbrb[:, :], gbr[:, :])
            nc.vector.tensor_copy(wwb[:, :], ww)

            with tc.tile_pool(name="dram", bufs=1, space="DRAM") as dp:
                cc_in = dp.tile([128, 4], f32)
                cc_out = dp.tile([128, 4], f32,
                                 addr_space="Shared" if n_cores > 1 else "Local")

                # bulk loads: x quarter0 -> l -> x rest -> xB
                xB, xb16B, free_xB = [], [], []
                xA, xb16A, free_xA = [], [], []
                for k in range(2):
                    t, fr = tc.tile([128, M1], f32, name=f"xB{k}")
                    xB.append(t)
                    free_xB.append(fr)
                    t, fr = tc.tile([128, M1], f16, name=f"xb16B{k}")
                    xb16B.append(t)
                    free_xB.append(fr)
                for k in range(2):
                    t, fr = tc.tile([128, M0], f32, name=f"xA{k}")
                    xA.append(t)
                    free_xA.append(fr)
                    t, fr = tc.tile([128, M0], f16, name=f"xb16A{k}")
                    xb16A.append(t)
                    free_xA.append(fr)
                l0, free_l0 = tc.tile([128, N], f32r, name="l0")
                l1, free_l1 = tc.tile([128, N], f32r, name="l1")
                lt0, free_lt0 = tc.tile([128, N], f32, name="lt0")
                lt1, free_lt1 = tc.tile([128, N], f32, name="lt1")
                nc.sync.dma_start(lt0[:, :], lres[0:128, :])
                for k in range(2):
                    nc.sync.dma_start(xA[k][:, 0:512], x[k * 128:(k + 1) * 128, 0:512])
                nc.sync.dma_start(lt1[:, :], lres[128:256, :])
                for k in range(2):
                    nc.sync.dma_start(xA[k][:, 512:M0],
                                      x[k * 128:(k + 1) * 128, 512:M0])
                for k in range(2):
                    nc.sync.dma_start(xB[k][:, :], x[k * 128:(k + 1) * 128, M0:N])
                # round l to f32r (phi's matmul operand) in the DMA shadow;
                # l1 split ACT/DVE so phi-k1 starts right behind it
                for c in range(4):
                    sl = slice(c * 1024, (c + 1) * 1024)
                    nc.scalar.activation(l0[:, sl], lt0[:, sl], AF.Identity,
                                         scale=1.0)
                nc.vector.tensor_copy(lb0[:, :], lt0[:, :])
                for c in range(4):
                    sl = slice(c * 1024, (c + 1) * 1024)
                    if c % 2 == 0:
                        nc.scalar.activation(l1[:, sl], lt1[:, sl], AF.Identity,
                                             scale=1.0)
                    else:
                        nc.vector.tensor_copy(l1[:, sl], lt1[:, sl])

                def theta_chunk(ps, c, xk, base, on_act):
                    pt = ps.tile([128, 1024], f32, tag="s", name=f"thps{base + c}")
                    sl = slice(base + c * 512, base + (c + 1) * 512)
                    for k in range(2):
                        nc.tensor.matmul(pt[:, 0:512],
                                         thwb[:, k * CI:(k + 1) * CI],
                                         xk[k][:, c * 512:(c + 1) * 512],
                                         start=(k == 0), stop=(k == 1))
                    if on_act:
                        nc.scalar.activation(theta[:, sl], pt[:, 0:512],
                                             AF.Identity, bias=thb, scale=1.0)
                    else:
                        nc.vector.tensor_scalar(theta[:, sl], pt[:, 0:512],
                                                thb, None, op0=ALU.add)

                def g_group(ps, t0, nt_cnt, xk, on_act):
                    # nt_cnt gT tiles batched into one PSUM slot, wide copies
                    pg = ps.tile([128, 1024], f32, tag="s", name=f"gps{t0}")
                    for j in range(nt_cnt):
                        nt = t0 + j
                        nsl = slice((nt * 128) % M0, (nt * 128) % M0 + 128)
                        osl = slice(j * 128, (j + 1) * 128)
                        nc.tensor.matmul(pg[:, osl], xk[0][:, nsl], gwb[:, 0:CI],
                                         start=True, stop=False)
                        nc.tensor.matmul(pg[:, osl], xk[1][:, nsl],
                                         gwb[:, CI:2 * CI], start=False, stop=False)
                        nc.tensor.matmul(pg[:, osl], onesb[:, :], gbrb[:, :],
                                         start=False, stop=True)
                    w = nt_cnt * 128
                    gsl = slice(t0 * 128, t0 * 128 + w)
                    if on_act:
                        nc.scalar.activation(gts[:, gsl], pg[:, 0:w],
                                             AF.Identity, scale=1.0)
                        nc.scalar.activation(g8[:, gsl], gts[:, gsl],
                                             AF.Identity, scale=1.0)
                    else:
                        nc.vector.tensor_copy(gts[:, gsl], pg[:, 0:w])

                # ---- head-1 (own PSUM pool, closed before the loop) ----
                with tc.tile_pool(name="ps0", bufs=4, space="PSUM") as ps0:
                    for k in range(2):
                        nc.vector.tensor_copy(xb16A[k][:, 0:512], xA[k][:, 0:512])
                    theta_chunk(ps0, 0, xb16A, 0, True)
                    g_group(ps0, 0, 4, xb16A, True)
                    # phi: ALL k0 partials first (run in l1's DMA shadow,
                    # re-emitted once as a PE p-state bridge), then all k1.
                    # Copies split ACT/DVE to shorten the loop-gating drain.
                    pts = []
                    for c in range(4):
                        pts.append(ps0.tile([128, 1024], f32, tag="s",
                                            name=f"phps{c}"))
                    # p-state bridge: complete throwaway groups first
                    for rep in range(2):
                        for c in range(4):
                            if rep == 1 and c >= 2:
                                continue
                            for h in range(2):
                                sl = slice(c * 1024 + h * 512,
                                           c * 1024 + (h + 1) * 512)
                                nc.tensor.matmul(
                                    pts[c][:, h * 512:(h + 1) * 512],
                                    phwr[:, 0:CI], l0[:, sl],
                                    start=True, stop=True)
                    for c in range(4):
                        for h in range(2):
                            sl = slice(c * 1024 + h * 512,
                                       c * 1024 + (h + 1) * 512)
                            nc.tensor.matmul(
                                pts[c][:, h * 512:(h + 1) * 512],
                                phwr[:, 0:CI], l0[:, sl],
                                start=True, stop=False)
                    nc.vector.tensor_copy(lb1[:, :], lt1[:, :])
                    for c in range(4):
                        for h in range(2):
                            sl = slice(c * 1024 + h * 512, c * 1024 + (h + 1) * 512)
                            nc.tensor.matmul(
                                pts[c][:, h * 512:(h + 1) * 512],
                                phwr[:, CI:2 * CI], l1[:, sl],
                                start=False, stop=True)
                        psl = slice(c * 1024, (c + 1) * 1024)
                        if c % 2 == 0:
                            nc.scalar.activation(phi[:, psl], pts[c][:, :],
                                                 AF.Identity, scale=1.0)
                        else:
                            nc.vector.tensor_copy(phi[:, psl], pts[c][:, :])
                free_lt1()
                free_lt0()
                free_l1()
                free_l0()

                # ---- attention n-loop (fstore outlives the loop pools) ----
                with tc.tile_pool(name="fstore", bufs=1) as fsp:
                  fstore = fsp.tile([128, NT * M1], f8)   # 64KB/part
                  with tc.tile_pool(name="psS", bufs=2, space="PSUM") as psS, \
                       tc.tile_pool(name="psY0", bufs=1, space="PSUM") as psY0, \
                       tc.tile_pool(name="zp", bufs=2) as zp, \
                       tc.tile_pool(name="fwp", bufs=2) as fwp, \
                       tc.tile_pool(name="ftp", bufs=2) as ftp:
                    y0 = psY0.tile([CI, M0], f32)
                    prev = None  # (g_nt ap, fw tile) pending y0 matmuls

                    def pre_trailing(nt):
                        # pure-DVE feed copies (no PSUM involvement)
                        if nt in (0, 1, 2):   # xb16A rest: 2x[128,512] per iter
                            for k in range(2):
                                sl = slice(512 + nt * 512, 1024 + nt * 512)
                                nc.gpsimd.tensor_copy(xb16A[k][:, sl],
                                                      xA[k][:, sl])
                        if 3 <= nt <= 6:      # xb16B k0/k1 in [128,1024] pieces
                            k, q = (nt - 3) % 2, (nt - 3) // 2
                            sl = slice(q * 1024, (q + 1) * 1024)
                            nc.gpsimd.tensor_copy(xb16B[k][:, sl], xB[k][:, sl])

                    def trailing(nt):
                        # at most ONE PSUM-slot borrower per iteration, with
                        # tiles batched 8-wide (gT) / 2-wide (theta) to halve
                        # the slot-rotation bubbles; emitted at iteration end
                        if nt == 1:       # gT tiles 4-7 (needed from iter 4)
                            g_group(psS, 4, 4, xb16A, False)
                        elif nt == 3:     # gT tiles 8-15 (needed from iter 8)
                            g_group(psS, 8, 8, xb16A, False)
                        elif nt == 5:     # gT tiles 16-23 (xb16B cols 0:1024)
                            g_group(psS, 16, 8, xb16B, False)
                        elif nt == 7:     # gT tiles 24-31
                            g_group(psS, 24, 8, xb16B, False)
                        elif nt in (2, 6, 8, 10):
                            # theta chunks, paired two-per-slot (contiguous
                            # in theta's m axis): (1,2) (3,4) (5,6) (7,)
                            g0 = {2: 1, 6: 3, 8: 5, 10: 7}[nt]
                            cnt = 1 if g0 == 7 else 2
                            pt = psS.tile([128, 1024], f32, tag="s",
                                          name=f"thp{g0}")
                            for j in range(cnt):
                                gc = g0 + j
                                xk = xb16A if gc < 4 else xb16B
                                csl = slice((gc % 4) * 512, (gc % 4) * 512 + 512)
                                for k in range(2):
                                    nc.tensor.matmul(
                                        pt[:, j * 512:(j + 1) * 512],
                                        thwb[:, k * CI:(k + 1) * CI],
                                        xk[k][:, csl],
                                        start=(k == 0), stop=(k == 1))
                            nc.vector.tensor_scalar(
                                theta[:, g0 * 512:(g0 + cnt) * 512],
                                pt[:, 0:cnt * 512], thb, None, op0=ALU.add)
                        if nt in (2, 4, 6, 8):
                            # fp8 copies of groups finished earlier, before
                            # any of their tiles get scaled in-place
                            t0 = {2: 4, 4: 8, 6: 16, 8: 24}[nt]
                            w = 512 if nt == 2 else 1024
                            gsl = slice(t0 * 128, t0 * 128 + w)
                            nc.vector.tensor_copy(g8[:, gsl], gts[:, gsl])

                    for nt in range(NT):
                        pre_trailing(nt)
                        th_nt = theta[:, nt * 128:(nt + 1) * 128]
                        fw = fwp.tile([128, M0], bf16, tag="fw", name=f"fw{nt}")
                        ft = ftp.tile([128, M1], bf16, tag="ft", name=f"ft{nt}")
                        for c in range(4):
                            sp = psS.tile([128, 1024], f32, tag="s")
                            for h in range(2):
                                sl = slice(c * 1024 + h * 512,
                                           c * 1024 + (h + 1) * 512)
                                nc.tensor.matmul(sp[:, h * 512:(h + 1) * 512],
                                                 th_nt, phi[:, sl],
                                                 start=True, stop=True)
                            if c < 2:
                                dst = fw[:, c * 1024:(c + 1) * 1024]
                            else:
                                dst = ft[:, (c - 2) * 1024:(c - 1) * 1024]
                            nc.scalar.activation(dst, sp[:, :], AF.Exp,
                                                 bias=negshift[:, :], scale=1.0)
                            # software-pipeline: previous iteration's y0
                            # matmuls interleave between S chunks
                            if prev is not None and c % 2 == 1:
                                pg_nt, pfw = prev
                                for q in range(2):
                                    qsl = slice((c // 2) * 1024 + q * 512,
                                                (c // 2) * 1024 + (q + 1) * 512)
                                    nc.tensor.matmul(
                                        y0[:, qsl], pg_nt, pfw[:, qsl],
                                        start=(nt == 1), stop=False)
                        # Z = rowsum(exp): fw half via DVE 4x tensor_scalar
                        # accum, ft half via Pool reduce (SBUF-only engine)
                        za = zp.tile([128, 2], f32, tag="z", name=f"z{nt}")
                        nc.vector.tensor_scalar(fw[:, :], fw[:, :], 0.0, None,
                                                op0=ALU.add, op1=ALU.add,
                                                accum_out=za[:, 0:1])
                        nc.vector.tensor_scalar(ft[:, :], ft[:, :], 0.0, None,
                                                op0=ALU.add, op1=ALU.add,
                                                accum_out=za[:, 1:2])
                        z = zp.tile([128, 1], f32, tag="zs", name=f"zs{nt}")
                        nc.vector.reduce_sum(z[:, :], za[:, :], axis=AX.X)
                        rz = zp.tile([128, 1], f32, tag="rz", name=f"rz{nt}")
                        nc.vector.reciprocal(rz[:, :], z[:, :])
                        g_nt = gts[:, nt * CI:(nt + 1) * CI]
                        nc.vector.tensor_scalar_mul(g_nt, g_nt, rz[:, :])
                        rzg = zp.tile([128, 1], f32, tag="rzg", name=f"rzg{nt}")
                        nc.vector.tensor_scalar_mul(rzg[:, :], rz[:, :], FP8G)
                        # normalized (gained) fp8 store of the second m-half
                        nc.gpsimd.tensor_scalar_mul(
                            fstore[:, nt * M1:(nt + 1) * M1], ft[:, :], rzg[:, :])
                        trailing(nt)
                        prev = (g_nt, fw)
                    # final iteration's y0 matmuls + drain
                    pg_nt, pfw = prev
                    for c in range(4):
                        qsl = slice(c * 512, (c + 1) * 512)
                        nc.tensor.matmul(y0[:, qsl], pg_nt, pfw[:, qsl],
                                         start=False, stop=True)
                    # preload sqrt table off the critical path (Identity/
                    # Square live in the sqrt set too; Exp is done)
                    nc.scalar.activation(statsg[:, 0:1], epsb[:, :], AF.Sqrt,
                                         scale=1.0)
                    nc.scalar.activation(ysb[:, 0:1024], y0[:, 0:1024],
                                         AF.Identity, scale=1.0)
                    nc.scalar.activation(ysb[:, 1024:2048], y0[:, 1024:2048],
                                         AF.Identity, scale=1.0)

                  # ---- phase 2: W conv + BN stats + y1 (fp8 DoubleRow) ----
                  with tc.tile_pool(name="psW", bufs=2, space="PSUM") as psW, \
                       tc.tile_pool(name="psY1", bufs=2, space="PSUM") as psY1, \
                       tc.tile_pool(name="s2p", bufs=2) as s2p:

                    def w_stats(cth, pc, pcol):
                        s2scr = s2p.tile([128, 1024], bf16, tag="sa",
                                         name=f"sa{cth}_{pc}")
                        s2scr2 = s2p.tile([128, 1024], bf16, tag="sb",
                                          name=f"sb{cth}_{pc}")
                        wsl = slice(cth * 128, (cth + 1) * 128)
                        wp = psW.tile([128, 1024], f32, tag="w")
                        for h in range(2):
                            sl = slice(pc * 1024 + h * 512,
                                       pc * 1024 + (h + 1) * 512)
                            nc.tensor.matmul(wp[:, h * 512:(h + 1) * 512],
                                             wwb[:, wsl], ysb[:, sl],
                                             start=True, stop=True)
                        # wy+w_b -> scratch; accum gives the S1 part
                        nc.scalar.activation(
                            s2scr[:, :], wp[:, :], AF.Identity,
                            bias=wb[:, cth:cth + 1], scale=1.0,
                            accum_out=s1c[:, cth * 4 + pcol:cth * 4 + pcol + 1])
                        # S2 part: square on DVE (2x bf16) + accum reduce
                        nc.vector.tensor_mul(s2scr2[:, :], s2scr[:, :],
                                             s2scr[:, :])
                        nc.vector.tensor_scalar(
                            s2scr[:, :], s2scr2[:, :], 0.0, None, op0=ALU.add,
                            op1=ALU.add,
                            accum_out=s2c[:, cth * 4 + pcol:cth * 4 + pcol + 1])

                    def y1_chunk(mc):
                        py = psY1.tile([128, 512], f32, tag="y1")
                        for t in range(NT // 2):
                            gpair = g8[:, 2 * t * CI:(2 * t + 2) * CI] \
                                .rearrange("p (two c) -> p two c", two=2)
                            fpair = fstore[:, 2 * t * M1:(2 * t + 2) * M1] \
                                .rearrange("p (two m) -> p two m", two=2)[
                                    :, :, mc * 512:(mc + 1) * 512]
                            nc.tensor.matmul(py[:, :], gpair, fpair,
                                             start=(t == 0),
                                             stop=(t == NT // 2 - 1),
                                             perf_mode=DR)
                        nc.scalar.activation(
                            ysb[:, M0 + mc * 512:M0 + (mc + 1) * 512],
                            py[:, :], AF.Identity, scale=1.0 / FP8G)

                    y1_chunk(0)
                    y1_chunk(1)
                    w_stats(0, 0, 0)
                    w_stats(1, 0, 0)
                    y1_chunk(2)
                    w_stats(0, 1, 1)
                    w_stats(1, 1, 1)
                    y1_chunk(3)
                    w_stats(0, 2, 2)
                    w_stats(1, 2, 2)
                    w_stats(0, 3, 3)
                    w_stats(1, 3, 3)
                    for cth in range(2):
                        nc.vector.reduce_sum(stats[:, cth:cth + 1],
                                             s1c[:, cth * 4:(cth + 1) * 4],
                                             axis=AX.X)
                        nc.vector.reduce_sum(stats[:, 2 + cth:3 + cth],
                                             s2c[:, cth * 4:(cth + 1) * 4],
                                             axis=AX.X)
                    if dbg:
                        nc.sync.dma_start(dbg["theta"][:, :], theta[:, :])
                        nc.sync.dma_start(dbg["phi"][:, :], phi[:, :])
                        nc.sync.dma_start(dbg["gts"][:, :], gts[:, :])
                        nc.sync.dma_start(dbg["g8"][:, :], g8[:, :])
                        nc.sync.dma_start(dbg["fstore"][:, :], fstore[:, :])
                        nc.sync.dma_start(dbg["ysb"][:, :], ysb[:, :])
                        nc.sync.dma_start(dbg["stats"][:, :], stats[:, :])

                for fr in reversed(free_xA):
                    fr()
                for fr in reversed(free_xB):
                    fr()

                # ---- all-reduce + finalize ----
                nc.sync.dma_start(cc_in[:, :], stats[:, :])
                if no_collective:
                    nc.sync.dma_start(cc_out[:, :], cc_in[:, :])
                else:
                    nc.gpsimd.collective_compute(
                        "AllReduce", mybir.AluOpType.add,
                        replica_groups=[list(range(n_cores))],
                        ins=[cc_in.opt()], outs=[cc_out.opt()])
                nc.sync.dma_start(statsg[:, :], cc_out[:, :])

                with tc.tile_pool(name="fin", bufs=1) as fp2, \
                     tc.tile_pool(name="obuf", bufs=3) as obp, \
                     tc.tile_pool(name="psF", bufs=2, space="PSUM") as psF:
                    inv = 1.0 / (B * N)
                    mean2 = fp2.tile([128, 2], f32)
                    e2 = fp2.tile([128, 2], f32)
                    var2 = fp2.tile([128, 2], f32)
                    sq = fp2.tile([128, 2], f32)
                    rstd = fp2.tile([128, 2], f32)
                    acol = fp2.tile([128, 2], f32)
                    btot = fp2.tile([128, 2], f32)
                    # keep PE at full p-state through the AR wait
                    for w in range(8):
                        nc.tensor.matmul(psF.tile([128, 2048], f32, tag="f",
                                                  name=f"warm{w}")[:, 0:512],
                                         wwb[:, 0:128], ysb[:, 0:512],
                                         start=True, stop=True)
                    nc.vector.tensor_scalar_mul(mean2[:, :], statsg[:, 0:2], inv)
                    nc.vector.tensor_scalar_mul(e2[:, :], statsg[:, 2:4], inv)
                    nc.vector.tensor_mul(var2[:, :], mean2[:, :], mean2[:, :])
                    nc.vector.tensor_sub(var2[:, :], e2[:, :], var2[:, :])
                    nc.scalar.activation(sq[:, :], var2[:, :], AF.Sqrt,
                                         bias=epsb[:, :], scale=1.0)
                    nc.vector.reciprocal(rstd[:, :], sq[:, :])
                    nc.vector.tensor_mul(acol[:, :], rstd[:, :], gam)
                    # recomputed conv excludes bias: btot = (w_b - mean)*a + beta
                    nc.vector.tensor_sub(btot[:, :], wb, mean2[:, :])
                    nc.vector.tensor_mul(btot[:, :], btot[:, :], acol[:, :])
                    nc.vector.tensor_add(btot[:, :], btot[:, :], bet)
                    # recompute W conv; normalize; + l; store
                    for cth in range(2):
                        wsl = slice(cth * 128, (cth + 1) * 128)
                        lt = lb0 if cth == 0 else lb1
                        for pc in range(2):
                            psl = slice(pc * 2048, (pc + 1) * 2048)
                            fpp = psF.tile([128, 2048], f32, tag="f")
                            for h in range(4):
                                sl = slice(pc * 2048 + h * 512,
                                           pc * 2048 + (h + 1) * 512)
                                nc.tensor.matmul(fpp[:, h * 512:(h + 1) * 512],
                                                 wwb[:, wsl], ysb[:, sl],
                                                 start=True, stop=True)
                            ob = obp.tile([128, 2048], bf16, tag="ob")
                            nc.scalar.activation(ob[:, :], fpp[:, :], AF.Identity,
                                                 bias=btot[:, cth:cth + 1],
                                                 scale=acol[:, cth:cth + 1])
                            o2 = obp.tile([128, 2048], f32, tag="o2")
                            if pc % 2 == 0:
                                nc.vector.tensor_add(o2[:, :], ob[:, :],
                                                     lt[:, psl])
                                nc.sync.dma_start(out[wsl, psl], o2[:, :])
                            else:
                                nc.gpsimd.tensor_add(o2[:, :], ob[:, :],
                                                     lt[:, psl])
                                nc.scalar.dma_start(out[wsl, psl], o2[:, :])

    nc.compile()
    return nc


def _get_nc(n_cores: int):
    if n_cores not in _CACHE:
        _CACHE[n_cores] = _build(n_cores)
    return _CACHE[n_cores]


def make_in_maps(inputs: dict, n_cores: int = N_CORES):
    """Build per-core input maps from full-size inputs."""
    f = np.float32
    x = np.ascontiguousarray(inputs["x"], f).reshape(B, CS, N)
    l = np.ascontiguousarray(inputs["l"], f).reshape(B, CT, N)
    pk = np.zeros((128, PP), f)
    thwT = np.asarray(inputs["theta_w"], f).T   # [CS, CI]
    phwT = np.asarray(inputs["phi_w"], f).T
    gwT = np.asarray(inputs["g_w"], f).T
    wwT = np.asarray(inputs["w_w"], f).T        # [CI, CT]
    pk[:, _THW:_THW + 128] = thwT[0:128]
    pk[:, _THW + 128:_THW + 256] = thwT[128:256]
    pk[:, _PHW:_PHW + 128] = phwT[0:128]
    pk[:, _PHW + 128:_PHW + 256] = phwT[128:256]
    pk[:, _GW:_GW + 128] = gwT[0:128]
    pk[:, _GW + 128:_GW + 256] = gwT[128:256]
    pk[:, _WW:_WW + 256] = wwT
    pk[:, _THB] = np.asarray(inputs["theta_b"], f)
    pk[:, _WB] = np.asarray(inputs["w_b"], f)[0:128]
    pk[:, _WB + 1] = np.asarray(inputs["w_b"], f)[128:256]
    pk[:, _GAM] = np.asarray(inputs["bn_gamma"], f)[0:128]
    pk[:, _GAM + 1] = np.asarray(inputs["bn_gamma"], f)[128:256]
    pk[:, _BET] = np.asarray(inputs["bn_beta"], f)[0:128]
    pk[:, _BET + 1] = np.asarray(inputs["bn_beta"], f)[128:256]
    shared = {
        "pk": pk,
        "g_b_row": np.ascontiguousarray(inputs["g_b"], f).reshape(1, CI),
    }
    return [{"x": x[i], "lres": l[i], **shared} for i in range(n_cores)]


def kernel(**inputs) -> np.ndarray:
    from concourse import bass_utils

    nc = _get_nc(N_CORES)
    in_maps = make_in_maps(inputs, N_CORES)
    res = bass_utils.run_bass_kernel_spmd(
        nc, in_maps, core_ids=list(range(N_CORES)))
    outs = [res.results[i]["out"] for i in range(N_CORES)]
    return np.stack(outs, 0).reshape(B, CT, 64, 64).astype(np.float32)


if __name__ == "__main__":
    nc = _get_nc(N_CORES)
    print("build+compile OK")
